# revision 32
# baseline (speedup 1.0000x reference)
"""Expert-parallel MoE FFN kernel for Trainium2 (8 NeuronCores, one expert per core).

Host side: routes tokens to experts (dedup per expert, summing duplicate top-k
weights), pads each expert's token list to a common T_PAD (multiple of 32,
sized to the max per-expert count), and pre-tiles the weight matrices into
DMA-friendly contiguous blocks.

Device side (per core, expert e):
  h^T = silu(G_e^T X^T) * (U_e^T X^T)     [I, T]   (stage A)
  y   = (h^T)^T @ D_e  * cw               [T, H]   (stage B; h^T tiles are the
                                           stationary operand so the cw combine
                                           becomes a per-partition scale)
All matmuls are bf16 with fp32 PSUM accumulation.

Perf structure:
 - Short HAM warm-up (dummy matmuls on an uninitialized SBUF region, first
   issue ~7.3us) opens the PE's HAM activity window ~3.4us before the first
   real matmul needs full clock.
 - During stage-A i=0 the kernel is input-DMA-bound, so the head inputs
   (x stream, gate0/up0 halves, gate/up i=1..3) go out in exact need order,
   each transfer split in half across the Sync and Scalar queues: one queue
   alone feeds the DGE rings at only ~2/3 rate, and separate queues run
   concurrently regardless of program order, so same-rank halves keep both
   feed rate AND need-order delivery. cw and the d tiles queue behind.
 - One shared PSUM pool (tags q0..q3, bufs=2) spans both stages, so stage B's
   accumulators take over stage A's bank ring without a pool barrier.
 - Stage B drains per 512-column PSUM chunk (cw scale on Scalar/Vector,
   immediate per-chunk y DMA on rotating queues). The last group runs its
   chunks' i-accumulations back-to-back (chunk-sequential) so chunk stops
   are ~3.5us apart and only the final chunk's drain sits past the last
   matmul. The NEFF's fixed semaphore-teardown epilogue starts right after
   the last y DMA, so tail ns are end-to-end ns.
 - A post-schedule pass drops LDWEIGHTS reloads whose stationary tile is
   already in the PE array, and another hoists prefetch-satisfied semaphore
   waits off PE instructions so LDWEIGHTS stays eligible for pull-ahead.
"""
import sys

if "/opt/trn_rl_repo" not in sys.path:
    sys.path.insert(0, "/opt/trn_rl_repo")

import numpy as np

N_TOKENS, TOP_K, N_EXPERTS, HIDDEN, INTER = 4096, 2, 8, 1024, 2048
P = 128
NI = INTER // P          # 16 I-tiles
KH = HIDDEN // P         # 8 H(contraction)-tiles
NHC = HIDDEN // 512      # 2 output-column chunks
N_WARM = 5               # HAM warm-up matmuls (N=512 each, ~0.43us cold)

_CACHE = {}


def _build(t_pad):
    import concourse.bacc as bacc
    import concourse.mybir as mybir
    import concourse.tile as tile

    f32 = mybir.dt.float32
    bf16 = mybir.dt.bfloat16

    nt = (t_pad + P - 1) // P          # t-blocks of 128 (last may be partial)
    # stage-A free-dim chunks of <=512 covering t_pad
    chunks = [(c, min(c + 512, t_pad)) for c in range(0, t_pad, 512)]
    ntc = len(chunks)
    assert ntc <= 2, "PSUM tag layout assumes at most 2 token chunks"

    nc = bacc.Bacc()
    xt = nc.declare_dram_parameter("xt", [KH, P, t_pad], bf16, isOutput=False)
    gw = nc.declare_dram_parameter("gw", [NI, P, HIDDEN], bf16, isOutput=False)
    uw = nc.declare_dram_parameter("uw", [NI, P, HIDDEN], bf16, isOutput=False)
    dw = nc.declare_dram_parameter("dw", [NI, P, HIDDEN], bf16, isOutput=False)
    cw = nc.declare_dram_parameter("cw", [P, nt], f32, isOutput=False)
    y = nc.declare_dram_parameter("y", [t_pad, HIDDEN], bf16, isOutput=True)

    with tile.TileContext(nc) as tc:
        with (
            tc.tile_pool(name="xp", bufs=1) as xp,
            tc.tile_pool(name="hp", bufs=1) as hp,
            tc.tile_pool(name="wp", bufs=4) as wp,
            tc.tile_pool(name="dp", bufs=1) as dp,
            tc.tile_pool(name="cp", bufs=1) as cp,
            tc.tile_pool(name="gp", bufs=4) as gp,
            tc.tile_pool(name="ep", bufs=3) as ep,
            tc.tile_pool(name="ps", bufs=2, space="PSUM") as ps,
        ):
            # ---- HAM warm-up: dummy matmuls on a raw (uninitialized) SBUF
            # region -- no memset, no producer, so the PE issues the first
            # warm-up matmul right at tile-context entry (~7.2us), opening
            # the PE's HAM activity window ~3.4us before the first real
            # matmul needs full clock. Garbage operands are harmless: the
            # warm PSUM bank's first real use is start=True, which resets
            # has_written and overwrites.
            scratch = nc.alloc_sbuf_tensor("warm_src", [P, 512], bf16)
            sap = scratch.ap()
            wps = ps.tile([P, 512], f32, tag="q0", name="warm")
            for _ in range(N_WARM):
                nc.tensor.matmul(out=wps[:], lhsT=sap[:, 0:P],
                                 rhs=sap[:], start=True, stop=True)

            # ---- Input tiles. Each x k-tile is TWO chunk tiles and each
            # head gate/up tile is TWO k-half tiles, so the two DMA halves
            # (one per engine queue) are separate tiles with their own
            # dependency tracking -- two queues writing one tile would race
            # (the consumer only waits on the program-order last writer).
            xcs = [[xp.tile([P, c1 - c0], bf16, tag=f"x{k}_{c}",
                            name=f"xt{k}_{c}")
                    for c, (c0, c1) in enumerate(chunks)] for k in range(KH)]

            def whalves(i):
                return ([wp.tile([P, 512], bf16, tag=f"g{i}{h}",
                                 name=f"gt{i}{h}") for h in range(2)],
                        [wp.tile([P, 512], bf16, tag=f"u{i}{h}",
                                 name=f"ut{i}{h}") for h in range(2)])

            def xap(k, c):
                return xcs[k][c][:]

            def wslice(ts, half_tiles, k):
                # k-slice [P, 128] of a [P, HIDDEN] weight tile (or its
                # split k-half tiles for the prefetched i<4)
                if half_tiles is not None:
                    return half_tiles[k // 4][:, (k % 4) * P:(k % 4 + 1) * P]
                return ts[:, k * P:(k + 1) * P]

            # ---- DMA issue order. During stage-A i=0 the kernel is input-
            # DMA-bound (~360GB/s demand ~= ring supply) and the hardware
            # rings round-robin across engine queues, so anything queued on
            # scalar/gpsimd steals bandwidth from the x stream 1:1. Hence:
            # only the two first-matmul-critical transfers run in parallel
            # (x0 chunk0 on Sync, gate0 half0 on Scalar); everything else
            # rides Sync in exact need-order, and all stage-B/late inputs
            # (cw, gate/up prefetch beyond i=3, d tiles) issue after x7.
            # Need-ordered head stream. A single engine queue feeds the 16
            # DGE rings at only ~2/3 rate (measured), and items on separate
            # queues run CONCURRENTLY (ring round-robin) regardless of
            # program order -- so every head transfer is split in half
            # across the Sync and Scalar queues at the same need rank: two
            # queues' worth of feed rate, with delivery in true need order.
            wpre = [whalves(i) for i in range(4)]
            g0s, u0s = wpre[0]
            # ranks: [(sync_dst, sync_src), (scalar_dst, scalar_src)]
            ranks = [[(xcs[0][0], xt[0][:, 0:512]), (g0s[0], gw[0][:, 0:512])]]
            if ntc > 1:
                ranks.append([(xcs[0][1], xt[0][:, 512:t_pad]),
                              (u0s[0], uw[0][:, 0:512])])
            else:
                ranks.append([(u0s[0], uw[0][:, 0:512]), None])
            for k in range(1, KH):
                pair = [(xcs[k][0], xt[k][:, 0:512])]
                if ntc > 1:
                    pair.append((xcs[k][1], xt[k][:, 512:t_pad]))
                ranks.append(pair)
                if k == 4:
                    ranks.append([(g0s[1], gw[0][:, 512:HIDDEN]),
                                  (u0s[1], uw[0][:, 512:HIDDEN])])
            for i in range(1, 4):
                gh, uh = wpre[i]
                ranks.append([(gh[0], gw[i][:, 0:512]), (gh[1], gw[i][:, 512:HIDDEN])])
                ranks.append([(uh[0], uw[i][:, 0:512]), (uh[1], uw[i][:, 512:HIDDEN])])
            for pair in ranks:
                for qi, item in enumerate(pair):
                    if item is None:
                        continue
                    dst, src = item
                    eng = nc.sync if qi == 0 else nc.scalar
                    eng.dma_start(out=dst[:], in_=src)
            # cw (stage-B-only input, 128 tiny descriptors) and the d tiles
            # queue on Sync BEHIND the head stream: engine queues execute in
            # queue order, so these only touch the rings once the
            # head-critical stream has drained.
            cwt = cp.tile([P, nt], f32, name="cwt")
            nc.sync.dma_start(out=cwt[:], in_=cw[:])
            dts = [dp.tile([P, HIDDEN], bf16, tag=f"d{i}", name=f"dt{i}")
                   for i in range(NI)]

            hts = [hp.tile([P, t_pad], bf16, tag=f"h{i}", name=f"ht{i}")
                   for i in range(NI)]

            # ---- Stage A: h^T[i] = silu(G^T X^T) * (U^T X^T), tiled over I ----
            for i in range(NI):
                ghalf = uhalf = None
                if i < 4:
                    (ghalf, uhalf), gt, ut = wpre[i], None, None
                else:
                    gt = wp.tile([P, HIDDEN], bf16, tag="g", name=f"gt{i}")
                    ut = wp.tile([P, HIDDEN], bf16, tag="u", name=f"ut{i}")
                    nc.sync.dma_start(out=gt[:], in_=gw[i])
                    nc.sync.dma_start(out=ut[:], in_=uw[i])
                nc.sync.dma_start(out=dts[i][:], in_=dw[i])
                pgs = [ps.tile([P, 512], f32, tag=f"q{c}", name=f"pg{i}_{c}")
                       for c in range(ntc)]
                pus = [ps.tile([P, 512], f32, tag=f"q{ntc + c}", name=f"pu{i}_{c}")
                       for c in range(ntc)]
                for k in range(KH):
                    lg = wslice(gt, ghalf, k)
                    lu = wslice(ut, uhalf, k)
                    for c, (c0, c1) in enumerate(chunks):
                        nc.tensor.matmul(out=pgs[c][:, :c1 - c0], lhsT=lg,
                                         rhs=xap(k, c),
                                         start=(k == 0), stop=(k == KH - 1))
                    for c, (c0, c1) in enumerate(chunks):
                        nc.tensor.matmul(out=pus[c][:, :c1 - c0], lhsT=lu,
                                         rhs=xap(k, c),
                                         start=(k == 0), stop=(k == KH - 1))
                for c, (c0, c1) in enumerate(chunks):
                    w = c1 - c0
                    sg = gp.tile([P, 512], f32, tag="sg", name="sg")
                    nc.scalar.activation(out=sg[:, :w], in_=pgs[c][:, :w],
                                         func=mybir.ActivationFunctionType.Silu)
                    nc.vector.tensor_mul(out=hts[i][:, c0:c1],
                                         in0=sg[:, :w], in1=pus[c][:, :w])

            # ---- Stage B: y[tb,:] = sum_i (h^T tile)^T @ D[i], scaled by cw.
            # h^T tiles are stationary; D rows stream. Output is y [T, H], so
            # cw is a per-partition scalar (Scalar-engine scale / DVE
            # tensor_scalar) applied per 512-column chunk as soon as that
            # chunk's accumulation stops, with an immediate per-chunk DMA.
            # The very last chunk drains as two 256-column pieces on
            # parallel engine pairs to halve the end-of-kernel tail.
            dma_engs = [nc.sync, nc.scalar, nc.gpsimd]
            ngroups = (nt + 1) // 2

            def drain_chunk(pys_t, ti, tb, r, hc, seng, deng):
                ybt = ep.tile([P, 512], bf16, tag=f"y{ti}{hc}",
                              name=f"ybt{ti}{hc}")
                if seng is nc.scalar:
                    nc.scalar.activation(
                        out=ybt[:r, :], in_=pys_t[hc][:r, :],
                        func=mybir.ActivationFunctionType.Copy,
                        scale=cwt[:r, tb:tb + 1])
                else:
                    seng.tensor_scalar_mul(
                        ybt[:r, :], pys_t[hc][:r, :], cwt[:r, tb:tb + 1])
                deng.dma_start(
                    out=y[tb * P:tb * P + r, hc * 512:(hc + 1) * 512],
                    in_=ybt[:r, :])

            for g in range(ngroups):
                tbs = [tb for tb in (2 * g, 2 * g + 1) if tb < nt]
                rows = [min(P, t_pad - tb * P) for tb in tbs]
                pys = [[ps.tile([P, 512], f32, tag=f"q{ti * NHC + hc}",
                                name=f"py{g}_{ti}_{hc}")
                        for hc in range(NHC)] for ti in range(len(tbs))]
                if g < ngroups - 1:
                    # i-outer order: one LDWEIGHTS covers NHC matmuls
                    for i in range(NI):
                        for ti, tb in enumerate(tbs):
                            r = rows[ti]
                            lh = hts[i][:, tb * P:tb * P + r]
                            for hc in range(NHC):
                                nc.tensor.matmul(
                                    out=pys[ti][hc][:r, :], lhsT=lh,
                                    rhs=dts[i][:, hc * 512:(hc + 1) * 512],
                                    start=(i == 0), stop=(i == NI - 1))
                    for ti, tb in enumerate(tbs):
                        r = rows[ti]
                        for hc in range(NHC):
                            drain_chunk(pys[ti], ti, tb, r, hc,
                                        nc.scalar if hc == 0 else nc.vector,
                                        dma_engs[(ti * NHC + hc) % 3])
                else:
                    # Last group runs chunk-SEQUENTIAL: each chunk's full
                    # i=0..15 accumulation back-to-back, so chunk stops are
                    # ~3.5us apart and every drain except the final chunk's
                    # overlaps remaining matmuls. The final chunk drains as
                    # two 64-row pieces on parallel engine pairs.
                    nchunks = len(tbs) * NHC
                    # only Scalar/Vector can read PSUM; chunk stops are
                    # ~3.5us apart so engine reuse is contention-free
                    sengs = [nc.scalar, nc.vector]
                    dengs = [nc.gpsimd, nc.gpsimd]
                    for ci in range(nchunks):
                        ti, hc = divmod(ci, NHC)
                        tb, r = tbs[ti], rows[ti]
                        lastc = ci == nchunks - 1
                        for i in range(NI):
                            nc.tensor.matmul(
                                out=pys[ti][hc][:r, :],
                                lhsT=hts[i][:, tb * P:tb * P + r],
                                rhs=dts[i][:, hc * 512:(hc + 1) * 512],
                                start=(i == 0), stop=(i == NI - 1))
                        drain_chunk(pys[ti], ti, tb, r, hc,
                                    sengs[ci % 2],
                                    nc.sync if lastc else dengs[ci % 2])

    _dedup_ldweights(nc)
    _hoist_pe_waits(nc)
    nc.finalize()
    return nc


def _hoist_pe_waits(nc, dist=12, skip=103):
    """Move semaphore waits off PE matmul/ldweights instructions into a
    standalone EVENT_SEMAPHORE `dist` engine-instructions earlier. A bare
    LDWEIGHTS can be pulled ahead of in-flight matmuls by the PE's reorder
    window; a wait-carrying one cannot (measured: 432ns vs 213ns pacing at
    every weight-ring boundary). All hoisted waits are prefetch-satisfied
    long before the insertion point. The first `skip` engine instructions
    (warm-up + first i-iteration, input DMA still in flight) keep their
    waits in place."""
    import concourse.mybir as mybir
    from collections import defaultdict

    for blk in nc.m.functions[0].blocks:
        pe_pos = [bi for bi, i in enumerate(blk.instructions)
                  if getattr(i, "engine", None) == mybir.EngineType.PE
                  and isinstance(i, (mybir.InstMatmult, mybir.InstLdweights))]
        if len(pe_pos) < skip:
            continue
        inserts = []  # (block_index, evsem)
        used_slots = set()
        for j, bi in enumerate(pe_pos):
            if j < skip:
                continue
            inst = blk.instructions[bi]
            si = inst.sync_info
            if not (si and si.on_wait):
                continue
            for w in si.on_wait:
                # DMA-completion waits (input prefetches, always long
                # satisfied) go ~`dist` engine-instructions early -- spread
                # so at most ONE hoisted EVSEM sits between any two PE
                # instructions (bunched EVSEMs serialize on the PE NX and
                # cost ~50ns each at every boundary). Engine-sem waits
                # (PSUM WAR etc.) split to an EVSEM immediately before the
                # instruction -- same queue position, but the LDW/MM itself
                # becomes bare and eligible for pull-ahead.
                if "DMA" in (w.ant_name or ""):
                    s = max(skip, j - dist)
                    while s in used_slots and s < j:
                        s += 1
                    if s >= j:
                        t = bi
                    else:
                        used_slots.add(s)
                        t = pe_pos[s]
                else:
                    t = bi
                ev = mybir.InstEventSemaphore(
                    name=nc.get_next_instruction_name(), ins=[], outs=[])
                ev.engine = mybir.EngineType.PE
                ev.sync_info = mybir.SyncInfo(on_wait=[w], on_update=[])
                nc.register_instruction(ev)
                inserts.append((t, ev))
            si.on_wait = []
        if not inserts:
            continue
        by_idx = defaultdict(list)
        for t, ev in inserts:
            by_idx[t].append(ev)
        out = []
        for bi, inst in enumerate(blk.instructions):
            if bi in by_idx:
                out.extend(by_idx[bi])
            out.append(inst)
        blk.instructions[:] = out


def _dedup_ldweights(nc):
    """Drop an InstLdweights whose weights AP matches the immediately
    preceding load on the PE queue (matmuls between don't clobber the
    array). Saves the ~46ns/matmul the redundant reload steals from the
    PE issue pipeline. Only sync-free duplicates are removed."""
    import concourse.mybir as mybir

    def key(i):
        a = i.ins[0]
        return (a.memref, a.offset, tuple(map(tuple, a.ap)), str(a.dtype),
                str(i.perf_mode), str(i.is_transpose),
                str(getattr(i, "tile_position", None)))

    for blk in nc.m.functions[0].blocks:
        last = None
        keep = []
        for i in blk.instructions:
            if getattr(i, "engine", None) == mybir.EngineType.PE:
                if isinstance(i, mybir.InstLdweights):
                    k = key(i)
                    si = i.sync_info
                    clean = not (si and (si.on_wait or si.on_update))
                    if k == last and clean:
                        continue
                    last = k
                elif not isinstance(i, mybir.InstMatmult):
                    last = None
            keep.append(i)
        blk.instructions[:] = keep


def _route(expert_indices, expert_weights):
    idx = np.asarray(expert_indices).astype(np.int64)
    wts = np.asarray(expert_weights).astype(np.float32)
    n = idx.shape[0]
    cw_full = np.zeros((N_EXPERTS, n), np.float32)
    for k in range(idx.shape[1]):
        np.add.at(cw_full, (idx[:, k], np.arange(n)), wts[:, k])
    ids = [np.nonzero(cw_full[e])[0] for e in range(N_EXPERTS)]
    maxc = max(len(i) for i in ids)
    t_pad = max(512, ((maxc + 31) // 32) * 32)
    return cw_full, ids, t_pad


def _run(nc, in_maps, trace=False, trace_cores=None):
    from concourse.bass_utils import run_bass_kernel_spmd

    return run_bass_kernel_spmd(
        nc, in_maps, list(range(N_EXPERTS)), trace=trace,
        trace_cores=trace_cores,
    )


def prepare(tokens, expert_indices, expert_weights, gate_weight, up_weight,
            down_weight):
    """Host-side routing + layout. Returns (nc, in_maps, ids, t_pad)."""
    tokens = np.ascontiguousarray(np.asarray(tokens, dtype=np.float32))
    gate_weight = np.asarray(gate_weight, dtype=np.float32)
    up_weight = np.asarray(up_weight, dtype=np.float32)
    down_weight = np.asarray(down_weight, dtype=np.float32)

    cw_full, ids, t_pad = _route(expert_indices, expert_weights)
    nt = (t_pad + P - 1) // P

    key = t_pad
    if key not in _CACHE:
        _CACHE[key] = _build(t_pad)
    nc = _CACHE[key]

    mmdt = np.dtype("bfloat16")
    in_maps = []
    for e in range(N_EXPERTS):
        ce = len(ids[e])
        xe = np.zeros((HIDDEN, t_pad), np.float32)
        xe[:, :ce] = tokens[ids[e]].T
        cwe = np.zeros((nt * P,), np.float32)
        cwe[:ce] = cw_full[e, ids[e]]
        in_maps.append({
            "xt": np.ascontiguousarray(xe.reshape(KH, P, t_pad)).astype(mmdt),
            "gw": np.ascontiguousarray(
                gate_weight[e].reshape(KH, P, NI, P).transpose(2, 1, 0, 3)
            ).reshape(NI, P, HIDDEN).astype(mmdt),
            "uw": np.ascontiguousarray(
                up_weight[e].reshape(KH, P, NI, P).transpose(2, 1, 0, 3)
            ).reshape(NI, P, HIDDEN).astype(mmdt),
            "dw": np.ascontiguousarray(down_weight[e].reshape(NI, P, HIDDEN)).astype(mmdt),
            "cw": np.ascontiguousarray(cwe.reshape(nt, P).T),
        })
    return nc, in_maps, ids, t_pad


def combine(results, ids):
    out = np.zeros((N_TOKENS, HIDDEN), np.float32)
    for e in range(N_EXPERTS):
        ce = len(ids[e])
        out[ids[e]] += results[e]["y"][:ce].astype(np.float32)
    return out


def kernel(tokens, expert_indices, expert_weights, gate_weight, up_weight,
           down_weight):
    nc, in_maps, ids, _ = prepare(tokens, expert_indices, expert_weights,
                                  gate_weight, up_weight, down_weight)
    res = _run(nc, in_maps, trace=False)
    return combine(res.results, ids)


# revision 37
# speedup vs baseline: 1.0030x; 1.0030x over previous
"""Expert-parallel MoE FFN kernel for Trainium2 (8 NeuronCores, one expert per core).

Host side: routes tokens to experts (dedup per expert, summing duplicate top-k
weights), pads each expert's token list to a common T_PAD (multiple of 32,
sized to the max per-expert count), and pre-tiles the weight matrices into
DMA-friendly contiguous blocks.

Device side (per core, expert e):
  h^T = silu(G_e^T X^T) * (U_e^T X^T)     [I, T]   (stage A)
  y   = (h^T)^T @ D_e  * cw               [T, H]   (stage B; h^T tiles are the
                                           stationary operand so the cw combine
                                           becomes a per-partition scale)
All matmuls are bf16 with fp32 PSUM accumulation.

Perf structure:
 - Short HAM warm-up (dummy matmuls on a GpSimd-memset scratch tile, first
   issue ~7.8us) opens the PE's HAM activity window ~3.4us before the first
   real matmul needs full clock.
 - During stage-A i=0 the kernel is input-DMA-bound, so the head inputs
   (x stream, gate0/up0 halves, gate/up i=1..3) go out in exact need order,
   each transfer split in half across the Sync and Scalar queues: one queue
   alone feeds the DGE rings at only ~2/3 rate, and separate queues run
   concurrently regardless of program order, so same-rank halves keep both
   feed rate AND need-order delivery. cw and the d tiles queue behind.
 - One shared PSUM pool (tags q0..q3, bufs=2) spans both stages, so stage B's
   accumulators take over stage A's bank ring without a pool barrier.
 - Stage B drains per 512-column PSUM chunk (cw scale on Scalar/Vector,
   immediate per-chunk y DMA on rotating queues). The last group runs its
   chunks' i-accumulations back-to-back (chunk-sequential) so chunk stops
   are ~3.5us apart and only the final chunk's drain sits past the last
   matmul. The NEFF's fixed semaphore-teardown epilogue starts right after
   the last y DMA, so tail ns are end-to-end ns.
 - A post-schedule pass drops LDWEIGHTS reloads whose stationary tile is
   already in the PE array, and another hoists prefetch-satisfied semaphore
   waits off PE instructions so LDWEIGHTS stays eligible for pull-ahead.
"""
import sys

if "/opt/trn_rl_repo" not in sys.path:
    sys.path.insert(0, "/opt/trn_rl_repo")

import numpy as np

N_TOKENS, TOP_K, N_EXPERTS, HIDDEN, INTER = 4096, 2, 8, 1024, 2048
P = 128
NI = INTER // P          # 16 I-tiles
KH = HIDDEN // P         # 8 H(contraction)-tiles
NHC = HIDDEN // 512      # 2 output-column chunks
N_WARM = 4               # HAM warm-up matmuls (N=512 each, ~0.43us cold)

_CACHE = {}


def _build(t_pad):
    import concourse.bacc as bacc
    import concourse.mybir as mybir
    import concourse.tile as tile

    f32 = mybir.dt.float32
    bf16 = mybir.dt.bfloat16

    nt = (t_pad + P - 1) // P          # t-blocks of 128 (last may be partial)
    # stage-A free-dim chunks of <=512 covering t_pad
    chunks = [(c, min(c + 512, t_pad)) for c in range(0, t_pad, 512)]
    ntc = len(chunks)
    assert ntc <= 2, "PSUM tag layout assumes at most 2 token chunks"

    nc = bacc.Bacc()
    xt = nc.declare_dram_parameter("xt", [KH, P, t_pad], bf16, isOutput=False)
    gw = nc.declare_dram_parameter("gw", [NI, P, HIDDEN], bf16, isOutput=False)
    uw = nc.declare_dram_parameter("uw", [NI, P, HIDDEN], bf16, isOutput=False)
    dw = nc.declare_dram_parameter("dw", [NI, P, HIDDEN], bf16, isOutput=False)
    cw = nc.declare_dram_parameter("cw", [P, nt], f32, isOutput=False)
    y = nc.declare_dram_parameter("y", [t_pad, HIDDEN], bf16, isOutput=True)

    with tile.TileContext(nc) as tc:
        with (
            tc.tile_pool(name="xp", bufs=1) as xp,
            tc.tile_pool(name="hp", bufs=1) as hp,
            tc.tile_pool(name="wp", bufs=4) as wp,
            tc.tile_pool(name="dp", bufs=1) as dp,
            tc.tile_pool(name="cp", bufs=1) as cp,
            tc.tile_pool(name="gp", bufs=4) as gp,
            tc.tile_pool(name="ep", bufs=3) as ep,
            tc.tile_pool(name="sp", bufs=1) as sp,
            tc.tile_pool(name="ps", bufs=2, space="PSUM") as ps,
        ):
            # ---- HAM warm-up: dummy matmuls on a GpSimd-memset scratch tile
            # (GpSimd is idle in the head, so the memset lands ~7.5us and
            # the first warm-up matmul ~7.8us), opening the PE's HAM
            # activity window ~3.4us before the first real matmul needs
            # full clock. The scratch MUST be initialized: leftover SBUF
            # garbage can be Inf/NaN bit patterns, which was observed to
            # intermittently corrupt results.
            scratch = sp.tile([P, 512], bf16, name="scratch")
            nc.gpsimd.memset(scratch[:], 0)
            wps = ps.tile([P, 512], f32, tag="q0", name="warm")
            for _ in range(N_WARM):
                nc.tensor.matmul(out=wps[:], lhsT=scratch[:, 0:P],
                                 rhs=scratch[:], start=True, stop=True)

            # ---- Input tiles. Each x k-tile is TWO chunk tiles and each
            # head gate/up tile is TWO k-half tiles, so the two DMA halves
            # (one per engine queue) are separate tiles with their own
            # dependency tracking -- two queues writing one tile would race
            # (the consumer only waits on the program-order last writer).
            xcs = [[xp.tile([P, c1 - c0], bf16, tag=f"x{k}_{c}",
                            name=f"xt{k}_{c}")
                    for c, (c0, c1) in enumerate(chunks)] for k in range(KH)]

            def whalves(i):
                return ([wp.tile([P, 512], bf16, tag=f"g{i}{h}",
                                 name=f"gt{i}{h}") for h in range(2)],
                        [wp.tile([P, 512], bf16, tag=f"u{i}{h}",
                                 name=f"ut{i}{h}") for h in range(2)])

            def xap(k, c):
                return xcs[k][c][:]

            def wslice(ts, half_tiles, k):
                # k-slice [P, 128] of a [P, HIDDEN] weight tile (or its
                # split k-half tiles for the prefetched i<4)
                if half_tiles is not None:
                    return half_tiles[k // 4][:, (k % 4) * P:(k % 4 + 1) * P]
                return ts[:, k * P:(k + 1) * P]

            # ---- DMA issue order. During stage-A i=0 the kernel is input-
            # DMA-bound (~360GB/s demand ~= ring supply) and the hardware
            # rings round-robin across engine queues, so anything queued on
            # scalar/gpsimd steals bandwidth from the x stream 1:1. Hence:
            # only the two first-matmul-critical transfers run in parallel
            # (x0 chunk0 on Sync, gate0 half0 on Scalar); everything else
            # rides Sync in exact need-order, and all stage-B/late inputs
            # (cw, gate/up prefetch beyond i=3, d tiles) issue after x7.
            # Need-ordered head stream. A single engine queue feeds the 16
            # DGE rings at only ~2/3 rate (measured), and items on separate
            # queues run CONCURRENTLY (ring round-robin) regardless of
            # program order -- so every head transfer is split in half
            # across the Sync and Scalar queues at the same need rank: two
            # queues' worth of feed rate, with delivery in true need order.
            wpre = [whalves(i) for i in range(4)]
            g0s, u0s = wpre[0]
            # ranks: [(sync_dst, sync_src), (scalar_dst, scalar_src)]
            ranks = [[(xcs[0][0], xt[0][:, 0:512]), (g0s[0], gw[0][:, 0:512])]]
            if ntc > 1:
                ranks.append([(xcs[0][1], xt[0][:, 512:t_pad]),
                              (u0s[0], uw[0][:, 0:512])])
            else:
                ranks.append([(u0s[0], uw[0][:, 0:512]), None])
            for k in range(1, KH):
                pair = [(xcs[k][0], xt[k][:, 0:512])]
                if ntc > 1:
                    pair.append((xcs[k][1], xt[k][:, 512:t_pad]))
                ranks.append(pair)
                if k == 4:
                    ranks.append([(g0s[1], gw[0][:, 512:HIDDEN]),
                                  (u0s[1], uw[0][:, 512:HIDDEN])])
            for i in range(1, 4):
                gh, uh = wpre[i]
                ranks.append([(gh[0], gw[i][:, 0:512]), (gh[1], gw[i][:, 512:HIDDEN])])
                ranks.append([(uh[0], uw[i][:, 0:512]), (uh[1], uw[i][:, 512:HIDDEN])])
            for pair in ranks:
                for qi, item in enumerate(pair):
                    if item is None:
                        continue
                    dst, src = item
                    eng = nc.sync if qi == 0 else nc.scalar
                    eng.dma_start(out=dst[:], in_=src)
            # cw (stage-B-only input, 128 tiny descriptors) and the d tiles
            # queue on Sync BEHIND the head stream: engine queues execute in
            # queue order, so these only touch the rings once the
            # head-critical stream has drained.
            cwt = cp.tile([P, nt], f32, name="cwt")
            nc.sync.dma_start(out=cwt[:], in_=cw[:])
            dts = [dp.tile([P, HIDDEN], bf16, tag=f"d{i}", name=f"dt{i}")
                   for i in range(NI)]

            hts = [hp.tile([P, t_pad], bf16, tag=f"h{i}", name=f"ht{i}")
                   for i in range(NI)]

            # ---- Stage A: h^T[i] = silu(G^T X^T) * (U^T X^T), tiled over I ----
            for i in range(NI):
                ghalf = uhalf = None
                if i < 4:
                    (ghalf, uhalf), gt, ut = wpre[i], None, None
                else:
                    gt = wp.tile([P, HIDDEN], bf16, tag="g", name=f"gt{i}")
                    ut = wp.tile([P, HIDDEN], bf16, tag="u", name=f"ut{i}")
                    nc.sync.dma_start(out=gt[:], in_=gw[i])
                    nc.sync.dma_start(out=ut[:], in_=uw[i])
                nc.sync.dma_start(out=dts[i][:], in_=dw[i])
                pgs = [ps.tile([P, 512], f32, tag=f"q{c}", name=f"pg{i}_{c}")
                       for c in range(ntc)]
                pus = [ps.tile([P, 512], f32, tag=f"q{ntc + c}", name=f"pu{i}_{c}")
                       for c in range(ntc)]
                for k in range(KH):
                    lg = wslice(gt, ghalf, k)
                    lu = wslice(ut, uhalf, k)
                    for c, (c0, c1) in enumerate(chunks):
                        nc.tensor.matmul(out=pgs[c][:, :c1 - c0], lhsT=lg,
                                         rhs=xap(k, c),
                                         start=(k == 0), stop=(k == KH - 1))
                    for c, (c0, c1) in enumerate(chunks):
                        nc.tensor.matmul(out=pus[c][:, :c1 - c0], lhsT=lu,
                                         rhs=xap(k, c),
                                         start=(k == 0), stop=(k == KH - 1))
                for c, (c0, c1) in enumerate(chunks):
                    w = c1 - c0
                    sg = gp.tile([P, 512], f32, tag="sg", name="sg")
                    nc.scalar.activation(out=sg[:, :w], in_=pgs[c][:, :w],
                                         func=mybir.ActivationFunctionType.Silu)
                    nc.vector.tensor_mul(out=hts[i][:, c0:c1],
                                         in0=sg[:, :w], in1=pus[c][:, :w])

            # ---- Stage B: y[tb,:] = sum_i (h^T tile)^T @ D[i], scaled by cw.
            # h^T tiles are stationary; D rows stream. Output is y [T, H], so
            # cw is a per-partition scalar (Scalar-engine scale / DVE
            # tensor_scalar) applied per 512-column chunk as soon as that
            # chunk's accumulation stops, with an immediate per-chunk DMA.
            # The very last chunk drains as two 256-column pieces on
            # parallel engine pairs to halve the end-of-kernel tail.
            dma_engs = [nc.sync, nc.scalar, nc.gpsimd]
            ngroups = (nt + 1) // 2

            def drain_chunk(pys_t, ti, tb, r, hc, seng, deng):
                ybt = ep.tile([P, 512], bf16, tag=f"y{ti}{hc}",
                              name=f"ybt{ti}{hc}")
                if seng is nc.scalar:
                    nc.scalar.activation(
                        out=ybt[:r, :], in_=pys_t[hc][:r, :],
                        func=mybir.ActivationFunctionType.Copy,
                        scale=cwt[:r, tb:tb + 1])
                else:
                    seng.tensor_scalar_mul(
                        ybt[:r, :], pys_t[hc][:r, :], cwt[:r, tb:tb + 1])
                deng.dma_start(
                    out=y[tb * P:tb * P + r, hc * 512:(hc + 1) * 512],
                    in_=ybt[:r, :])

            for g in range(ngroups):
                tbs = [tb for tb in (2 * g, 2 * g + 1) if tb < nt]
                rows = [min(P, t_pad - tb * P) for tb in tbs]
                pys = [[ps.tile([P, 512], f32, tag=f"q{ti * NHC + hc}",
                                name=f"py{g}_{ti}_{hc}")
                        for hc in range(NHC)] for ti in range(len(tbs))]
                if g < ngroups - 1:
                    # i-outer order: one LDWEIGHTS covers NHC matmuls
                    for i in range(NI):
                        for ti, tb in enumerate(tbs):
                            r = rows[ti]
                            lh = hts[i][:, tb * P:tb * P + r]
                            for hc in range(NHC):
                                nc.tensor.matmul(
                                    out=pys[ti][hc][:r, :], lhsT=lh,
                                    rhs=dts[i][:, hc * 512:(hc + 1) * 512],
                                    start=(i == 0), stop=(i == NI - 1))
                    for ti, tb in enumerate(tbs):
                        r = rows[ti]
                        for hc in range(NHC):
                            drain_chunk(pys[ti], ti, tb, r, hc,
                                        nc.scalar if hc == 0 else nc.vector,
                                        dma_engs[(ti * NHC + hc) % 3])
                else:
                    # Last group runs chunk-SEQUENTIAL: each chunk's full
                    # i=0..15 accumulation back-to-back, so chunk stops are
                    # ~3.5us apart and every drain except the final chunk's
                    # overlaps remaining matmuls. The final chunk drains as
                    # two 64-row pieces on parallel engine pairs.
                    nchunks = len(tbs) * NHC
                    # only Scalar/Vector can read PSUM; chunk stops are
                    # ~3.5us apart so engine reuse is contention-free
                    sengs = [nc.scalar, nc.vector]
                    dengs = [nc.gpsimd, nc.gpsimd]
                    for ci in range(nchunks):
                        ti, hc = divmod(ci, NHC)
                        tb, r = tbs[ti], rows[ti]
                        lastc = ci == nchunks - 1
                        for i in range(NI):
                            nc.tensor.matmul(
                                out=pys[ti][hc][:r, :],
                                lhsT=hts[i][:, tb * P:tb * P + r],
                                rhs=dts[i][:, hc * 512:(hc + 1) * 512],
                                start=(i == 0), stop=(i == NI - 1))
                        drain_chunk(pys[ti], ti, tb, r, hc,
                                    sengs[ci % 2],
                                    nc.sync if lastc else dengs[ci % 2])

    _dedup_ldweights(nc)
    _hoist_pe_waits(nc)
    nc.finalize()
    return nc


def _hoist_pe_waits(nc, dist=8, skip=54):
    """Move semaphore waits off PE matmul/ldweights instructions into a
    standalone EVENT_SEMAPHORE `dist` engine-instructions earlier. A bare
    LDWEIGHTS can be pulled ahead of in-flight matmuls by the PE's reorder
    window; a wait-carrying one cannot (measured: 432ns vs 213ns pacing at
    every weight-ring boundary). All hoisted waits are prefetch-satisfied
    long before the insertion point. The first `skip` engine instructions
    (warm-up + first i-iteration, input DMA still in flight) keep their
    waits in place."""
    import concourse.mybir as mybir
    from collections import defaultdict

    for blk in nc.m.functions[0].blocks:
        pe_pos = [bi for bi, i in enumerate(blk.instructions)
                  if getattr(i, "engine", None) == mybir.EngineType.PE
                  and isinstance(i, (mybir.InstMatmult, mybir.InstLdweights))]
        if len(pe_pos) < skip:
            continue
        inserts = []  # (block_index, evsem)
        used_slots = set()
        for j, bi in enumerate(pe_pos):
            if j < skip:
                continue
            inst = blk.instructions[bi]
            si = inst.sync_info
            if not (si and si.on_wait):
                continue
            for w in si.on_wait:
                # DMA-completion waits (input prefetches, always long
                # satisfied) go ~`dist` engine-instructions early -- spread
                # so at most ONE hoisted EVSEM sits between any two PE
                # instructions (bunched EVSEMs serialize on the PE NX and
                # cost ~50ns each at every boundary). Engine-sem waits
                # (PSUM WAR etc.) split to an EVSEM immediately before the
                # instruction -- same queue position, but the LDW/MM itself
                # becomes bare and eligible for pull-ahead.
                if "DMA" in (w.ant_name or ""):
                    s = max(skip, j - dist)
                    while s in used_slots and s < j:
                        s += 1
                    if s >= j:
                        t = bi
                    else:
                        used_slots.add(s)
                        t = pe_pos[s]
                else:
                    t = bi
                ev = mybir.InstEventSemaphore(
                    name=nc.get_next_instruction_name(), ins=[], outs=[])
                ev.engine = mybir.EngineType.PE
                ev.sync_info = mybir.SyncInfo(on_wait=[w], on_update=[])
                nc.register_instruction(ev)
                inserts.append((t, ev))
            si.on_wait = []
        if not inserts:
            continue
        by_idx = defaultdict(list)
        for t, ev in inserts:
            by_idx[t].append(ev)
        out = []
        for bi, inst in enumerate(blk.instructions):
            if bi in by_idx:
                out.extend(by_idx[bi])
            out.append(inst)
        blk.instructions[:] = out


def _dedup_ldweights(nc):
    """Drop an InstLdweights whose weights AP matches the immediately
    preceding load on the PE queue (matmuls between don't clobber the
    array). Saves the ~46ns/matmul the redundant reload steals from the
    PE issue pipeline. Only sync-free duplicates are removed."""
    import concourse.mybir as mybir

    def key(i):
        a = i.ins[0]
        return (a.memref, a.offset, tuple(map(tuple, a.ap)), str(a.dtype),
                str(i.perf_mode), str(i.is_transpose),
                str(getattr(i, "tile_position", None)))

    for blk in nc.m.functions[0].blocks:
        last = None
        keep = []
        for i in blk.instructions:
            if getattr(i, "engine", None) == mybir.EngineType.PE:
                if isinstance(i, mybir.InstLdweights):
                    k = key(i)
                    si = i.sync_info
                    clean = not (si and (si.on_wait or si.on_update))
                    if k == last and clean:
                        continue
                    last = k
                elif not isinstance(i, mybir.InstMatmult):
                    last = None
            keep.append(i)
        blk.instructions[:] = keep


def _route(expert_indices, expert_weights):
    idx = np.asarray(expert_indices).astype(np.int64)
    wts = np.asarray(expert_weights).astype(np.float32)
    n = idx.shape[0]
    cw_full = np.zeros((N_EXPERTS, n), np.float32)
    for k in range(idx.shape[1]):
        np.add.at(cw_full, (idx[:, k], np.arange(n)), wts[:, k])
    ids = [np.nonzero(cw_full[e])[0] for e in range(N_EXPERTS)]
    maxc = max(len(i) for i in ids)
    t_pad = max(512, ((maxc + 31) // 32) * 32)
    return cw_full, ids, t_pad


def _run(nc, in_maps, trace=False, trace_cores=None):
    from concourse.bass_utils import run_bass_kernel_spmd

    return run_bass_kernel_spmd(
        nc, in_maps, list(range(N_EXPERTS)), trace=trace,
        trace_cores=trace_cores,
    )


def prepare(tokens, expert_indices, expert_weights, gate_weight, up_weight,
            down_weight):
    """Host-side routing + layout. Returns (nc, in_maps, ids, t_pad)."""
    tokens = np.ascontiguousarray(np.asarray(tokens, dtype=np.float32))
    gate_weight = np.asarray(gate_weight, dtype=np.float32)
    up_weight = np.asarray(up_weight, dtype=np.float32)
    down_weight = np.asarray(down_weight, dtype=np.float32)

    cw_full, ids, t_pad = _route(expert_indices, expert_weights)
    nt = (t_pad + P - 1) // P

    key = t_pad
    if key not in _CACHE:
        _CACHE[key] = _build(t_pad)
    nc = _CACHE[key]

    mmdt = np.dtype("bfloat16")
    in_maps = []
    for e in range(N_EXPERTS):
        ce = len(ids[e])
        xe = np.zeros((HIDDEN, t_pad), np.float32)
        xe[:, :ce] = tokens[ids[e]].T
        cwe = np.zeros((nt * P,), np.float32)
        cwe[:ce] = cw_full[e, ids[e]]
        in_maps.append({
            "xt": np.ascontiguousarray(xe.reshape(KH, P, t_pad)).astype(mmdt),
            "gw": np.ascontiguousarray(
                gate_weight[e].reshape(KH, P, NI, P).transpose(2, 1, 0, 3)
            ).reshape(NI, P, HIDDEN).astype(mmdt),
            "uw": np.ascontiguousarray(
                up_weight[e].reshape(KH, P, NI, P).transpose(2, 1, 0, 3)
            ).reshape(NI, P, HIDDEN).astype(mmdt),
            "dw": np.ascontiguousarray(down_weight[e].reshape(NI, P, HIDDEN)).astype(mmdt),
            "cw": np.ascontiguousarray(cwe.reshape(nt, P).T),
        })
    return nc, in_maps, ids, t_pad


def combine(results, ids):
    out = np.zeros((N_TOKENS, HIDDEN), np.float32)
    for e in range(N_EXPERTS):
        ce = len(ids[e])
        out[ids[e]] += results[e]["y"][:ce].astype(np.float32)
    return out


def kernel(tokens, expert_indices, expert_weights, gate_weight, up_weight,
           down_weight):
    nc, in_maps, ids, _ = prepare(tokens, expert_indices, expert_weights,
                                  gate_weight, up_weight, down_weight)
    res = _run(nc, in_maps, trace=False)
    return combine(res.results, ids)
